# revision 1
# baseline (speedup 1.0000x reference)
import numpy as np
import ml_dtypes

# ---- problem constants (hardcoded from spec) ----
B, C, H, W = 2, 128, 256, 512
P = B * H * W               # 262144 pixels
TEMPERATURE = 0.1
BASE_TEMPERATURE = 0.07
MAX_SAMPLES = 1024
MAX_VIEWS = 100
NUM_CLASSES = 8
BIG_NEG = 1e9
N = NUM_CLASSES * MAX_SAMPLES   # 8192 sampled rows
N_CORES = 8
BLK = N // N_CORES              # 1024 rows/columns per core
SCALE = np.float32(BASE_TEMPERATURE / (TEMPERATURE * TEMPERATURE))  # 7.0f exactly
NEG_DIAG = -1.0e8               # exp(7*(g-1e8)) == 0 in f32, matches reference

_PROGRAM = {}


def _sample_indices_host(labels_flat_np):
    """Verbatim replication of reference._sample_indices on jax-CPU."""
    import jax
    import jax.numpy as jnp

    cpu = jax.devices("cpu")[0]
    with jax.default_device(cpu):
        labels_flat = jnp.asarray(labels_flat_np)
        key = jax.random.key(42)
        k1, k2 = jax.random.split(key)
        scores = jax.random.uniform(k1, (P,))
        class_mask = (
            labels_flat[None, :]
            == jnp.arange(NUM_CLASSES, dtype=labels_flat.dtype)[:, None]
        )
        masked_scores = jnp.where(class_mask, scores[None, :], -1.0)
        _, idx = jax.lax.top_k(masked_scores, MAX_SAMPLES)
        sampled_idx = idx.reshape(-1)
        row_scores = jax.random.uniform(k2, (N, MAX_SAMPLES))
        _, sel = jax.lax.top_k(row_scores, MAX_VIEWS)
        block_start = (jnp.arange(N) // MAX_SAMPLES) * MAX_SAMPLES
        pos_cols = sel + block_start[:, None]
        return np.asarray(sampled_idx), np.asarray(pos_cols)


NK = 5                  # cyclic block-columns computed per core (k = 0..4)
KC = NK * BLK           # 5120 columns of embR actually needed per core


def _build_program():
    """Build the Bass/Tile SPMD program once (shared by all 8 cores).

    Symmetry scheme: exp(7*G) is symmetric, so each core only computes its
    row block against cyclic column blocks k=0..4 (5/8 of the matrix).
    Row sums (== col sums of the transposed blocks) come free via ACT
    accum_out; for k=1..3 the per-column sums of the computed block are also
    needed (they complete other cores' k=5,6,7 row sums) and are produced by
    ones-vector matmuls accumulated in PSUM across the 8 row-chunks."""
    if _PROGRAM:
        return _PROGRAM

    import concourse.mybir as mybir
    from concourse import bacc, tile

    f32 = mybir.dt.float32
    bf16 = mybir.dt.bfloat16

    # Bacc (not plain Bass): its finalize() runs move_matmul_waits_to_ldweights
    # + generate_event_semaphores, which keep every Matmult within the 1
    # sync-wait HW limit.
    nc = bacc.Bacc("TRN2", target_bir_lowering=False)

    # embR: row-normalized embeddings, transposed [C, N], rolled so this
    # core's own 1024-column class block sits at columns 0..1023; only the
    # first KC=5120 columns are needed under the symmetry scheme.
    embR_d = nc.dram_tensor("embR", [128, KC], bf16, kind="ExternalInput")
    # eyes[:, 0:128] = I, eyes[:, 128:256] = -1e8 * I, eyes[:, 256] = ones
    eyes_d = nc.dram_tensor("eyes", [128, 264], bf16, kind="ExternalInput")

    rowsum_d = nc.dram_tensor("rowsum_parts", [128, 32], f32, kind="ExternalOutput")
    cs_d = nc.dram_tensor("cs_parts", [1, 3 * BLK], f32, kind="ExternalOutput")

    with tile.TileContext(nc) as tc:
        with (
            tc.tile_pool(name="persist", bufs=1) as persist,
            tc.tile_pool(name="work", bufs=2) as work,
        ):
            embR = persist.tile([128, KC], bf16)
            eyes = persist.tile([128, 264], bf16)
            rowsum_parts = persist.tile([128, 32], f32)
            cs_sb = persist.tile([1, 3 * BLK], f32)

            nc.sync.dma_start(out=eyes[:], in_=eyes_d[:])
            # phase A (k=1..3) runs first: it needs lhsT (cols 0:1024) and
            # cols 1024:2048 up front; the k0/k4 columns stream in later.
            emb_cuts = [(0, 512), (1024, 1536), (1536, 2048), (512, 1024),
                        (2048, 3072), (3072, 4096), (4096, KC)]
            for lo, hi in emb_cuts:
                nc.sync.dma_start(out=embR[:, lo:hi], in_=embR_d[:, lo:hi])

            ones_lhsT = eyes[:, 256:257]

            def emit_cs(cs_t, e_t, c):
                for b in range(2):
                    nc.tensor.matmul(
                        cs_t[:, b * 512:(b + 1) * 512],
                        ones_lhsT,
                        e_t[:, b * 512:(b + 1) * 512],
                        start=(c == 0),
                        stop=(c == 7),
                    )

            # ---- phase A: k = 1..3, 1024-wide sweeps + colsum accumulators
            with tc.tile_pool(name="psumA", bufs=2, space="PSUM") as psum:
                for k in (1, 2, 3):
                    cs_t = psum.tile([1, BLK], f32, tag="cs")
                    pend = None  # (e_tile, c) with deferred colsum matmuls
                    for c in range(8):
                        lhsT = embR[:, c * 128:(c + 1) * 128]
                        ps = psum.tile([128, BLK], f32, tag="ps")
                        for t in range(2):
                            nc.tensor.matmul(
                                ps[:, t * 512:(t + 1) * 512],
                                lhsT,
                                embR[:, k * BLK + t * 512: k * BLK + (t + 1) * 512],
                                start=True,
                                stop=True,
                            )
                        e = work.tile([128, BLK], bf16, tag="e")
                        idx = (k - 1) * 8 + c
                        nc.scalar.activation(
                            e[:],
                            ps[:],
                            mybir.ActivationFunctionType.Exp,
                            scale=float(SCALE),
                            accum_out=rowsum_parts[:, idx:idx + 1],
                        )
                        # defer this sweep's colsum matmuls one sweep so PE
                        # never stalls waiting on the ACT exp it consumes
                        if pend is not None:
                            emit_cs(cs_t, *pend)
                        pend = (e, c)
                    emit_cs(cs_t, *pend)
                    # drain closed colsum accumulator on the idle DVE
                    nc.vector.tensor_copy(
                        out=cs_sb[:, (k - 1) * BLK: k * BLK], in_=cs_t[:]
                    )
                nc.sync.dma_start(out=cs_d[:], in_=cs_sb[:])

            # ---- phase B: k=0 and k=4 fused into 2048-wide sweeps (no cs
            # needed, so both PSUM buffers can be full width: 0.96 vs 1.46
            # ns/elem on ACT)
            with tc.tile_pool(name="psumB", bufs=2, space="PSUM") as psum2:
                for c in range(8):
                    lhsT = embR[:, c * 128:(c + 1) * 128]
                    ps = psum2.tile([128, 2 * BLK], f32, tag="ps2")
                    td = c // 4  # 512-chunk holding this row-chunk's diagonal
                    for t in range(4):
                        col0 = t * 512 if t < 2 else 4 * BLK + (t - 2) * 512
                        is_diag_chunk = t == td
                        nc.tensor.matmul(
                            ps[:, t * 512:(t + 1) * 512],
                            lhsT,
                            embR[:, col0:col0 + 512],
                            start=True,
                            stop=not is_diag_chunk,
                        )
                        if is_diag_chunk:
                            # add -1e8 * I at columns 128c..128c+128
                            nc.tensor.matmul(
                                ps[:, c * 128:(c + 1) * 128],
                                eyes[:, 0:128],
                                eyes[:, 128:256],
                                start=False,
                                stop=True,
                            )
                    e = work.tile([128, 2 * BLK], bf16, tag="e2")
                    nc.scalar.activation(
                        e[:],
                        ps[:],
                        mybir.ActivationFunctionType.Exp,
                        scale=float(SCALE),
                        accum_out=rowsum_parts[:, 24 + c: 25 + c],
                    )

            nc.sync.dma_start(out=rowsum_d[:], in_=rowsum_parts[:])

    nc.finalize()
    _PROGRAM["nc"] = nc
    return _PROGRAM


def _spos_host(emb_n, pos_cols):
    """s_pos = sum of exp(7*dot) over all (row, pos) pairs, excluding
    self-pairs (the -7e8 diagonal makes their exp exactly 0 in f32)."""
    rows = np.repeat(np.arange(N), MAX_VIEWS)
    cols = pos_cols.ravel()
    mask = cols != rows
    rows, cols = rows[mask], cols[mask]
    total = 0.0
    for ofs in range(0, rows.size, 131072):
        r = rows[ofs:ofs + 131072]
        c = cols[ofs:ofs + 131072]
        dots = np.einsum("ij,ij->i", emb_n[r], emb_n[c], dtype=np.float64)
        total += float(np.exp(np.float64(SCALE) * dots).sum())
    return total


def _host_prep(embeddings, labels):
    sampled_idx, pos_cols = _sample_indices_host(labels.reshape(-1))
    hw = H * W
    b = sampled_idx // hw
    h = (sampled_idx % hw) // W
    w = sampled_idx % W
    emb_s = embeddings[b, :, h, w].astype(np.float32)  # [N, C]
    norm = np.sqrt(np.sum(emb_s * emb_s, axis=1, dtype=np.float32)).astype(np.float32)
    norm = np.maximum(norm, np.float32(1e-12))
    emb_n = emb_s / norm[:, None]
    embT = np.ascontiguousarray(emb_n.T).astype(ml_dtypes.bfloat16)  # [C, N]

    spos = _spos_host(emb_n, pos_cols)

    eyes = np.zeros((128, 264), dtype=ml_dtypes.bfloat16)
    eyes[:, 0:128] = np.eye(128, dtype=ml_dtypes.bfloat16)
    eyes[:, 128:256] = (NEG_DIAG * np.eye(128, dtype=np.float32)).astype(
        ml_dtypes.bfloat16
    )
    eyes[:, 256:264] = ml_dtypes.bfloat16(1.0)

    in_maps = []
    for m in range(N_CORES):
        embR = np.ascontiguousarray(np.roll(embT, -BLK * m, axis=1)[:, :KC])
        in_maps.append({"embR": embR, "eyes": eyes})
    return in_maps, spos


def _combine(results, spos_total):
    # per core r: row-sum over its k=0..4 blocks, [1024] indexed u = c*128+p
    rp, cs = [], []
    for res in results:
        parts = np.asarray(res["rowsum_parts"], dtype=np.float64)  # [128, 40]
        rp.append(parts.reshape(128, 4, 8).sum(axis=1).T.reshape(-1))
        cs.append(np.asarray(res["cs_parts"], dtype=np.float64).reshape(3, BLK))
    col_sum = np.empty(N, dtype=np.float64)
    for r in range(N_CORES):
        # missing blocks (r, r+5/6/7) are transposes of computed k'=3/2/1
        # blocks on cores r+5/6/7; their row-sum pieces are those cores'
        # column sums (cs slot k'-1), aligned at u = j - 1024r by the roll.
        col_sum[r * BLK:(r + 1) * BLK] = (
            rp[r]
            + cs[(r + 5) % 8][2]
            + cs[(r + 6) % 8][1]
            + cs[(r + 7) % 8][0]
        )
    loss = -np.log(spos_total) + np.mean(np.log(col_sum))
    return np.float32(loss)


def kernel(embeddings: np.ndarray, labels: np.ndarray) -> np.ndarray:
    from concourse.bass_utils import run_bass_kernel_spmd

    prog = _build_program()
    in_maps, spos = _host_prep(np.asarray(embeddings), np.asarray(labels))
    out = run_bass_kernel_spmd(prog["nc"], in_maps, list(range(N_CORES)))
    return _combine(out.results, spos)



# revision 4
# speedup vs baseline: 1.0765x; 1.0765x over previous
import numpy as np
import ml_dtypes

# ---- problem constants (hardcoded from spec) ----
B, C, H, W = 2, 128, 256, 512
P = B * H * W               # 262144 pixels
TEMPERATURE = 0.1
BASE_TEMPERATURE = 0.07
MAX_SAMPLES = 1024
MAX_VIEWS = 100
NUM_CLASSES = 8
BIG_NEG = 1e9
N = NUM_CLASSES * MAX_SAMPLES   # 8192 sampled rows
N_CORES = 8
BLK = N // N_CORES              # 1024 rows/columns per core
SCALE = np.float32(BASE_TEMPERATURE / (TEMPERATURE * TEMPERATURE))  # 7.0f exactly

_PROGRAM = {}


def _sample_indices_host(labels_flat_np):
    """Verbatim replication of reference._sample_indices on jax-CPU."""
    import jax
    import jax.numpy as jnp

    cpu = jax.devices("cpu")[0]
    with jax.default_device(cpu):
        labels_flat = jnp.asarray(labels_flat_np)
        key = jax.random.key(42)
        k1, k2 = jax.random.split(key)
        scores = jax.random.uniform(k1, (P,))
        class_mask = (
            labels_flat[None, :]
            == jnp.arange(NUM_CLASSES, dtype=labels_flat.dtype)[:, None]
        )
        masked_scores = jnp.where(class_mask, scores[None, :], -1.0)
        _, idx = jax.lax.top_k(masked_scores, MAX_SAMPLES)
        sampled_idx = idx.reshape(-1)
        row_scores = jax.random.uniform(k2, (N, MAX_SAMPLES))
        _, sel = jax.lax.top_k(row_scores, MAX_VIEWS)
        block_start = (jnp.arange(N) // MAX_SAMPLES) * MAX_SAMPLES
        pos_cols = sel + block_start[:, None]
        return np.asarray(sampled_idx), np.asarray(pos_cols)


NK = 5                  # cyclic block-columns computed per core (k = 0..4)
KC = NK * BLK           # 5120 columns of embR actually needed per core


def _build_program():
    """Bass/Tile SPMD program (shared by all 8 cores).

    Symmetry scheme: exp(7*G) is symmetric; each core computes its 1024-row
    block against cyclic column blocks k=0..4 (5/8 of the matrix).  The
    diagonal is NOT suppressed on device — the host subtracts exp(7*g_jj)
    (replicating the bf16 quantization) from the combined column sums.

    Per chunk c (128 rows), three PSUM tiles of 2048 columns each are
    matmul'd and exp'd by ACT into a persistent SBUF e-arena laid out in
    three sections: [k0k1 | k2k3 | k4].  k4 tiles of two adjacent chunks
    share one 2048-wide ACT.  A DVE scalar_tensor_tensor (4x bf16 mode)
    accumulates e into csacc per section and emits running row-sum
    accumulators; the host recovers per-chunk row sums by telescoping
    differences.  Column sums of the k1..k3 sections (csacc[:,1024:4096])
    are partition-summed on the host."""
    if _PROGRAM:
        return _PROGRAM

    import concourse.mybir as mybir
    from concourse import bacc, tile

    f32 = mybir.dt.float32
    bf16 = mybir.dt.bfloat16
    Alu = mybir.AluOpType

    nc = bacc.Bacc("TRN2", target_bir_lowering=False)

    # embR: row-normalized embeddings, transposed [C, N], rolled so this
    # core's own 1024-column class block sits at columns 0..1023.
    embR_d = nc.dram_tensor("embR", [128, KC], bf16, kind="ExternalInput")
    cs_d = nc.dram_tensor("cs", [128, 3 * BLK], bf16, kind="ExternalOutput")
    accs_d = nc.dram_tensor("accs", [128, 24], f32, kind="ExternalOutput")

    with tile.TileContext(nc) as tc:
        with (
            tc.tile_pool(name="persist", bufs=1) as persist,
            tc.tile_pool(name="psum", bufs=2, space="PSUM") as psum,
        ):
            embR = persist.tile([128, KC], bf16)
            earena = persist.tile([128, 8 * KC], bf16)   # 80KB/partition
            csacc = persist.tile([128, KC], bf16)
            accA = persist.tile([128, 8], f32)
            accB1 = persist.tile([128, 8], f32)
            accB2 = persist.tile([128, 8], f32)

            # stream embR in; first cut unblocks the first matmul quickly
            emb_cuts = [(0, 512), (512, 1024), (1024, 2048),
                        (2048, 3072), (3072, 4096), (4096, KC)]
            for lo, hi in emb_cuts:
                nc.sync.dma_start(out=embR[:, lo:hi], in_=embR_d[:, lo:hi])

            # section base offsets in earena / csacc column space
            S_K01 = 0            # 2048 per chunk  (cols 0:2048 of embR)
            S_K23 = 8 * 2048     # 2048 per chunk  (cols 2048:4096)
            S_K4 = 8 * 4096      # 1024 per chunk  (cols 4096:5120)

            def mm_tile(ps, c, col0):
                """4 matmuls filling ps[128,2048] = rows of chunk c x embR
                cols [col0, col0+2048)."""
                lhsT = embR[:, c * 128:(c + 1) * 128]
                for t in range(4):
                    nc.tensor.matmul(
                        ps[:, t * 512:(t + 1) * 512],
                        lhsT,
                        embR[:, col0 + t * 512: col0 + (t + 1) * 512],
                        start=True, stop=True,
                    )

            # zero the csacc sections up front (DVE is idle during the DMA
            # prologue; each section is zeroed before its first accumulate)
            nc.vector.memset(csacc[:, 0:2048], 0.0)
            nc.vector.memset(csacc[:, 2048:4096], 0.0)
            nc.vector.memset(csacc[:, 4096:KC], 0.0)

            def dve_acc(sec_lo, width, c, acc, e_ap):
                """csacc[sec] += e (bf16, DVE 4x) with running-sum accum."""
                cs_ap = csacc[:, sec_lo:sec_lo + width]
                nc.vector.scalar_tensor_tensor(
                    out=cs_ap, in0=e_ap, scalar=0.0, in1=cs_ap,
                    op0=Alu.add, op1=Alu.add, accum_out=acc[:, c:c + 1],
                )

            for pair in range(4):
                c0, c1 = 2 * pair, 2 * pair + 1
                for c in (c0, c1):
                    # T1: k0k1
                    ps = psum.tile([128, 2048], f32, tag="ps")
                    mm_tile(ps, c, 0)
                    e_ap = earena[:, S_K01 + c * 2048: S_K01 + (c + 1) * 2048]
                    nc.scalar.activation(
                        e_ap, ps[:], mybir.ActivationFunctionType.Exp,
                        scale=float(SCALE),
                    )
                    dve_acc(0, 2048, c, accA, e_ap)
                    # T2: k2k3
                    ps = psum.tile([128, 2048], f32, tag="ps")
                    mm_tile(ps, c, 2048)
                    e_ap = earena[:, S_K23 + c * 2048: S_K23 + (c + 1) * 2048]
                    nc.scalar.activation(
                        e_ap, ps[:], mybir.ActivationFunctionType.Exp,
                        scale=float(SCALE),
                    )
                    dve_acc(2048, 2048, c, accB1, e_ap)
                    if c == 7:
                        # k1..k3 column sums final except k4 section; start
                        # streaming them out under the remaining compute
                        nc.sync.dma_start(
                            out=cs_d[:, 0:1024], in_=csacc[:, 1024:2048]
                        )
                        nc.sync.dma_start(
                            out=cs_d[:, 1024:3072], in_=csacc[:, 2048:4096]
                        )
                # T5: k4 for both chunks of the pair, one 2048-wide ACT
                ps = psum.tile([128, 2048], f32, tag="ps")
                for i, c in enumerate((c0, c1)):
                    lhsT = embR[:, c * 128:(c + 1) * 128]
                    for t in range(2):
                        nc.tensor.matmul(
                            ps[:, i * 1024 + t * 512: i * 1024 + (t + 1) * 512],
                            lhsT,
                            embR[:, 4096 + t * 512: 4096 + (t + 1) * 512],
                            start=True, stop=True,
                        )
                e_ap = earena[:, S_K4 + c0 * 1024: S_K4 + (c1 + 1) * 1024]
                nc.scalar.activation(
                    e_ap, ps[:], mybir.ActivationFunctionType.Exp,
                    scale=float(SCALE),
                )
                for c in (c0, c1):
                    dve_acc(4096, 1024, c, accB2,
                            earena[:, S_K4 + c * 1024: S_K4 + (c + 1) * 1024])

            nc.sync.dma_start(out=accs_d[:, 0:8], in_=accA[:])
            nc.sync.dma_start(out=accs_d[:, 8:16], in_=accB1[:])
            nc.sync.dma_start(out=accs_d[:, 16:24], in_=accB2[:])

    nc.finalize()
    _PROGRAM["nc"] = nc
    return _PROGRAM


def _spos_host(emb_n, pos_cols):
    """s_pos = sum of exp(7*dot) over all (row, pos) pairs, excluding
    self-pairs (suppressed to exactly 0 in the reference)."""
    rows = np.repeat(np.arange(N), MAX_VIEWS)
    cols = pos_cols.ravel()
    mask = cols != rows
    rows, cols = rows[mask], cols[mask]
    total = 0.0
    for ofs in range(0, rows.size, 131072):
        r = rows[ofs:ofs + 131072]
        c = cols[ofs:ofs + 131072]
        dots = np.einsum("ij,ij->i", emb_n[r], emb_n[c], dtype=np.float64)
        total += float(np.exp(np.float64(SCALE) * dots).sum())
    return total


def _host_prep(embeddings, labels):
    sampled_idx, pos_cols = _sample_indices_host(labels.reshape(-1))
    hw = H * W
    b = sampled_idx // hw
    h = (sampled_idx % hw) // W
    w = sampled_idx % W
    emb_s = embeddings[b, :, h, w].astype(np.float32)  # [N, C]
    norm = np.sqrt(np.sum(emb_s * emb_s, axis=1, dtype=np.float32)).astype(np.float32)
    norm = np.maximum(norm, np.float32(1e-12))
    emb_n = emb_s / norm[:, None]
    embT = np.ascontiguousarray(emb_n.T).astype(ml_dtypes.bfloat16)  # [C, N]

    spos = _spos_host(emb_n, pos_cols)

    # diagonal correction: exp(7 * g_jj) with the same bf16 quantization the
    # device matmul sees
    q = embT.astype(np.float64)
    diag_e = np.exp(np.float64(SCALE) * (q * q).sum(axis=0))  # [N]

    in_maps = []
    for m in range(N_CORES):
        embR = np.ascontiguousarray(np.roll(embT, -BLK * m, axis=1)[:, :KC])
        in_maps.append({"embR": embR})
    return in_maps, (spos, diag_e)


def _combine(results, host_data):
    spos, diag_e = host_data
    rowsums, cs_k = [], []
    for res in results:
        accs = np.asarray(res["accs"], dtype=np.float64)  # [128, 24]
        rs = np.zeros((128, 8))
        for X in (accs[:, 0:8], accs[:, 8:16], accs[:, 16:24]):
            rs += np.diff(np.concatenate([np.zeros((128, 1)), X], axis=1), axis=1)
        rowsums.append(rs.T.reshape(-1))  # [1024], u = c*128 + p
        cs_k.append(np.asarray(res["cs"], dtype=np.float64).sum(axis=0))  # [3072]
    col_sum = np.empty(N, dtype=np.float64)
    for bblk in range(N_CORES):
        col_sum[bblk * BLK:(bblk + 1) * BLK] = (
            rowsums[bblk]
            + cs_k[(bblk - 1) % N_CORES][0:1024]
            + cs_k[(bblk - 2) % N_CORES][1024:2048]
            + cs_k[(bblk - 3) % N_CORES][2048:3072]
            - diag_e[bblk * BLK:(bblk + 1) * BLK]
        )
    loss = -np.log(spos) + np.mean(np.log(col_sum))
    return np.float32(loss)


def kernel(embeddings: np.ndarray, labels: np.ndarray) -> np.ndarray:
    from concourse.bass_utils import run_bass_kernel_spmd

    prog = _build_program()
    in_maps, host_data = _host_prep(np.asarray(embeddings), np.asarray(labels))
    out = run_bass_kernel_spmd(prog["nc"], in_maps, list(range(N_CORES)))
    return _combine(out.results, host_data)
